# revision 11
# baseline (speedup 1.0000x reference)
"""Int16 Conv1x1 Q8.8 kernel for 8x Trainium2 NeuronCores.

Problem: y = dequant(clip(rshift_round(int16_gemm(quant(x), w_q), 8) + b_q))
  x [8, 512, 4096] fp32, w_q [512, 512] int16, b_q [512] int16 -> y [8, 512, 4096] fp32

Sharding: data-parallel over batch B=8, one batch element per core; weights
replicated. No collectives.

Math: y = (W_q @ x)/256 + b_q/256 computed in fp16 (w_q ints and b_q/256 are
exact in fp16; x cast to fp16 on host). Rel err 1.5e-3 vs the 2e-2 gate.
fp8/int8 double-pumping rejected: the exact integer split needs 2 GEMMs at
~half-time each plus doubled LDWEIGHTS cost - net >= fp16 single GEMM.

HW model (from NTFF traces): exec is measured [framework-preamble end ~6.0us,
postamble end], and ~8.5us of serial semaphore teardown after the last DMA
queue quiesces is fixed framework cost. So exec ~= (last y byte + completion
sem) - 6.0us + 8.5us, and the levers are: first-matmul time, stall-free warm
PE window (27.7us floor for this fp16 GEMM), and last-drain->last-byte tail.

DMA reality (measured): the 16 DMA engines collectively service descriptor
LINES (one per partition per transfer) at ~165ns/line/engine regardless of
queue count - queues only parallelize the ~0.65us/transfer descriptor WRITE,
and engines drain descriptors in global generation order. So:
  - the head tile is minimal-LINE: one 128-line transfer [w_m0|w_m1|bias|
    x0(256 cols)] (4KB/line) -> lands ~1.5us after flow start; w_m2/m3 and
    x1 follow in FIFO order just-in-time behind the m-loop.
  - mid-stream x and y ride PAIRED chunks (8KB lines) to halve line count.
  - the last two chunks' y goes out per-m-subtile right after each drain
    (the only part of y that cannot ship earlier), so the post-matmul tail
    is just the final m3 piece: drain + issue + 128 small lines + sem.
Clock ramp: the PE runs 1.2GHz until ~3.4us of sustained activity (HAM), and
~1-2us idle gaps re-throttle it. Prewarm matmuls (128-wide, on a DVE-memset
dummy; DVE's completion-sem latency beats gpsimd's ~1.2us) bridge from the
barrier exit to the head tile's arrival so real matmuls start warm.
"""

from contextlib import ExitStack

import numpy as np

import concourse.bass as bass
import concourse.tile as tile
from concourse import bacc, mybir
from concourse.bass_utils import run_bass_kernel_spmd

F32 = mybir.dt.float32
F16 = mybir.dt.float16

P = 128
CIN = 512
COUT = 512
L = 4096
B = 8
KO = CIN // P          # 4 k-subtiles
MO = COUT // P         # 4 m-subtiles
NT = 512               # max free dim per matmul / psum bank
Q = 256.0
CHUNKS = [256, 512, 512, 512, 512, 512, 512, 512, 256]
OFFS = np.cumsum([0] + CHUNKS).tolist()
NCH = len(CHUNKS)
# x DMA groups: chunk indices per transfer (contiguous col ranges)
XGRP = [[1], [2, 3], [4, 5], [6, 7], [8]]
# y DMA groups for the full-rate chunks (pairs halve DMA line count)
YGRP = [[0], [1, 2], [3, 4], [5, 6]]
PREWARM = 16           # dummy matmuls bridging [barrier end, head landing]
PWFREE = 256
X0N = KO * CHUNKS[0]   # x0 elems per partition line
T1N = MO * KO * P + MO + X0N  # all w + bias + x0 per line

_cached_nc = None


def _build():
    nc = bacc.Bacc("TRN2", target_bir_lowering=False, debug=False, num_devices=B)

    t1_d = nc.dram_tensor("t1", [P, T1N], F16, kind="ExternalInput").ap()
    xg_d = {g[0]: nc.dram_tensor(f"xg{g[0]}", [P, KO * sum(CHUNKS[c] for c in g)],
                                 F16, kind="ExternalInput").ap() for g in XGRP}
    yg_d = {g[0]: nc.dram_tensor(f"yg{g[0]}", [P, MO * sum(CHUNKS[c] for c in g)],
                                 F16, kind="ExternalOutput").ap() for g in YGRP}
    # last two chunks ship per m-pair: [m0m1] and [m2m3]
    y7_ds = [nc.dram_tensor(f"y7h{h}", [P, 2 * CHUNKS[7]], F16,
                            kind="ExternalOutput").ap() for h in range(2)]
    y8_ds = [nc.dram_tensor(f"y8h{h}", [P, 2 * CHUNKS[8]], F16,
                            kind="ExternalOutput").ap() for h in range(2)]

    with tile.TileContext(nc) as tc, ExitStack() as ctx:
        dpool = ctx.enter_context(tc.tile_pool(name="d", bufs=1))
        wpool = ctx.enter_context(tc.tile_pool(name="w", bufs=1))
        xpool = ctx.enter_context(tc.tile_pool(name="x", bufs=len(XGRP)))
        ypool = ctx.enter_context(tc.tile_pool(name="y", bufs=4))
        pspool = ctx.enter_context(tc.tile_pool(name="ps", bufs=8, space="PSUM"))

        # PE prewarm: garbage matmuls gated only on a cheap DVE memset
        dmy = dpool.tile([P, PWFREE], F16)
        nc.vector.memset(dmy[:], 0.0)
        for _ in range(PREWARM):
            dps = pspool.tile([P, NT], F32, name="dps", tag="ps")
            nc.tensor.matmul(dps[:, :PWFREE], dmy[:, :P], dmy[:],
                             start=True, stop=True)

        # --- inputs: all on the sync HWDGE queue, exact consumption order ---
        t1_sb = wpool.tile([P, T1N], F16)
        wav = t1_sb[:, 0:MO * KO * P].rearrange("p (mo ko j) -> p mo ko j",
                                                mo=MO, ko=KO)
        cb16 = t1_sb[:, MO * KO * P:MO * KO * P + MO]
        x0v = t1_sb[:, MO * KO * P + MO:].rearrange("p (ko n) -> p ko n", ko=KO)
        cb = wpool.tile([P, MO], F32)

        xgt = {g[0]: xpool.tile([P, KO, sum(CHUNKS[c] for c in g)], F16,
                                tag="xt", name=f"xg{g[0]}") for g in XGRP}
        # chunk -> (tile, col offset within tile)
        xmap = {}
        for g in XGRP:
            off = 0
            for c in g:
                xmap[c] = (xgt[g[0]], off)
                off += CHUNKS[c]

        nc.sync.dma_start(t1_sb[:], t1_d)
        for g in XGRP:
            nc.sync.dma_start(xgt[g[0]][:], xg_d[g[0]].rearrange(
                "p (ko n) -> p ko n", ko=KO))
        nc.vector.tensor_scalar_add(cb[:], cb16, 0.0)

        ygt = {}
        for g in YGRP:
            ygt[g[0]] = ypool.tile([P, MO, sum(CHUNKS[c] for c in g)], F16,
                                   tag="yt", name=f"yg{g[0]}")
        yt7 = ypool.tile([P, MO, CHUNKS[7]], F16, tag="yt", name="yt7")
        yt8 = ypool.tile([P, MO, CHUNKS[8]], F16, tag="yt", name="yt8")
        ymap = {}
        for g in YGRP:
            off = 0
            for c in g:
                ymap[c] = (ygt[g[0]], off)
                off += CHUNKS[c]
        ymap[7] = (yt7, 0)
        ymap[8] = (yt8, 0)

        for c in range(NCH):
            wc = CHUNKS[c]
            yt, yoff = ymap[c]
            for m in range(MO):
                ps = pspool.tile([P, NT], F32, name="ps", tag="ps")
                for k in range(KO):
                    if c == 0:
                        rhs = x0v[:, k]
                    else:
                        xt, xoff = xmap[c]
                        rhs = xt[:, k, xoff:xoff + wc]
                    nc.tensor.matmul(ps[:, :wc], wav[:, m, k], rhs,
                                     start=(k == 0), stop=(k == KO - 1))
                # drain: y = ps/256 + b. DVE/ACT alternate; on the last chunk
                # m3 ends on DVE while the HWDGE engines write the final DMAs.
                ydst = yt[:, m, yoff:yoff + wc]
                use_dve = (c + m) % 2 == 0 if c < 8 else m in (0, 3)
                if use_dve:
                    nc.vector.tensor_scalar(ydst, ps[:, :wc],
                                            1.0 / Q, cb[:, m, None],
                                            mybir.AluOpType.mult,
                                            mybir.AluOpType.add)
                else:
                    nc.scalar.activation(ydst, ps[:, :wc],
                                         mybir.ActivationFunctionType.Identity,
                                         bias=cb[:, m, None], scale=1.0 / Q)
                # last two chunks ship per m-pair right after the later drain
                if m in (1, 3):
                    h = m // 2
                    if c == 7:
                        nc.gpsimd.dma_start(y7_ds[h], yt[:, 2 * h:2 * h + 2, :])
                    elif c == 8:
                        if h == 0:
                            nc.sync.dma_start(y8_ds[0], yt[:, 0:2, :])
                        else:
                            nc.sync.dma_start(y8_ds[1][0:64], yt[0:64, 2:4, :])
                            nc.scalar.dma_start(y8_ds[1][64:128],
                                                yt[64:128, 2:4, :])
            if c in (0, 2, 4, 6):
                g0 = c - 1 if c > 0 else 0
                gyt = ygt[g0]
                gd = yg_d[g0].rearrange("p (mo n) -> p mo n", mo=MO)
                nc.gpsimd.dma_start(gd, gyt[:])

    nc.compile()
    return nc


def _prep_in_maps(x, w_q, b_q):
    # int16 weights up to +-2048 and b_q/256 (11 significand bits) are
    # exact in fp16
    wT = w_q.T.reshape(KO, P, MO, P).transpose(1, 0, 2, 3)  # [p, ko, mo, 128]
    # per line: [m0k0..k3 | m1k0..k3 | m2... | m3...] then bias then x0
    wa = np.ascontiguousarray(wT.transpose(0, 2, 1, 3).reshape(P, MO * KO * P)
                              ).astype(np.float16)
    cb16 = (b_q.reshape(MO, P).T.astype(np.float32) / np.float32(Q)
            ).astype(np.float16)
    x16 = x.astype(np.float16)                                    # [B, Cin, L]
    xt = x16.reshape(B, KO, P, L).transpose(0, 2, 1, 3)           # [B, p, ko, l]
    maps = []
    for i in range(B):
        x0 = xt[i, :, :, OFFS[0]:OFFS[1]].reshape(P, X0N)
        m = {"t1": np.ascontiguousarray(np.concatenate(
                [wa, cb16, x0], axis=1))}
        for g in XGRP:
            lo, hi = OFFS[g[0]], OFFS[g[-1] + 1]
            m[f"xg{g[0]}"] = np.ascontiguousarray(
                xt[i, :, :, lo:hi]).reshape(P, KO * (hi - lo))
        maps.append(m)
    return maps


def kernel(x: np.ndarray, w_q: np.ndarray, b_q: np.ndarray) -> np.ndarray:
    global _cached_nc
    if _cached_nc is None:
        _cached_nc = _build()
    nc = _cached_nc

    in_maps = _prep_in_maps(x, w_q, b_q)
    res = run_bass_kernel_spmd(nc, in_maps, core_ids=list(range(B)))

    out = np.empty((B, COUT, L), dtype=np.float32)
    for i, r in enumerate(res.results):
        for g in YGRP:
            lo, hi = OFFS[g[0]], OFFS[g[-1] + 1]
            # [p, mo, n] -> [mo*128+p, lo:hi]
            yc = r[f"yg{g[0]}"].reshape(P, MO, hi - lo).transpose(1, 0, 2)
            out[i, :, lo:hi] = yc.reshape(COUT, hi - lo)
        for h in range(2):
            y7 = r[f"y7h{h}"].reshape(P, 2, CHUNKS[7]).transpose(1, 0, 2)
            out[i, 2 * h * P:(2 * h + 2) * P, OFFS[7]:OFFS[8]] = \
                y7.reshape(2 * P, CHUNKS[7])
            y8 = r[f"y8h{h}"].reshape(P, 2, CHUNKS[8]).transpose(1, 0, 2)
            out[i, 2 * h * P:(2 * h + 2) * P, OFFS[8]:] = \
                y8.reshape(2 * P, CHUNKS[8])
    return out


# revision 18
# speedup vs baseline: 1.0493x; 1.0493x over previous
"""Int16 Conv1x1 Q8.8 kernel for 8x Trainium2 NeuronCores.

Problem: y = dequant(clip(rshift_round(int16_gemm(quant(x), w_q), 8) + b_q))
  x [8, 512, 4096] fp32, w_q [512, 512] int16, b_q [512] int16 -> y [8, 512, 4096] fp32

Sharding: data-parallel over batch B=8, one batch element per core; weights
replicated. No collectives.

Math: y = (W_q @ x)/256 + b_q/256 computed in fp16 (w_q ints and b_q/256 are
exact in fp16; x cast to fp16 on host). Rel err 1.5e-3 vs the 2e-2 gate.
fp8/int8 double-pumping rejected: the exact integer split needs 2 GEMMs at
~half-time each plus doubled LDWEIGHTS cost - net >= fp16 single GEMM.

HW model (from NTFF traces): exec is measured [framework-preamble end ~6.0us,
postamble end], and ~8.5us of serial semaphore teardown after the last DMA
queue quiesces is fixed framework cost. So exec ~= (last y byte + completion
sem) - 6.0us + 8.5us, and the levers are: first-matmul time, stall-free warm
PE window (27.7us floor for this fp16 GEMM), and last-drain->last-byte tail.

DMA reality (measured): the 16 DMA engines collectively service descriptor
LINES (one per partition per transfer) at ~165ns/line/engine regardless of
queue count - queues only parallelize the ~0.65us/transfer descriptor WRITE,
and engines drain descriptors in global generation order. So:
  - the head tile is minimal-LINE: one 128-line transfer [w_m0|w_m1|bias|
    x0(256 cols)] (4KB/line) -> lands ~1.5us after flow start; w_m2/m3 and
    x1 follow in FIFO order just-in-time behind the m-loop.
  - mid-stream x and y ride PAIRED chunks (8KB lines) to halve line count.
  - the last two chunks' y goes out per-m-subtile right after each drain
    (the only part of y that cannot ship earlier), so the post-matmul tail
    is just the final m3 piece: drain + issue + 128 small lines + sem.
Clock ramp: the PE runs 1.2GHz until ~3.4us of sustained activity (HAM), and
~1-2us idle gaps re-throttle it. Prewarm matmuls (128-wide, on a DVE-memset
dummy; DVE's completion-sem latency beats gpsimd's ~1.2us) bridge from the
barrier exit to the head tile's arrival so real matmuls start warm.
"""

from contextlib import ExitStack

import numpy as np

import concourse.bass as bass
import concourse.tile as tile
from concourse import bacc, mybir
from concourse.bass_utils import run_bass_kernel_spmd

F32 = mybir.dt.float32
F16 = mybir.dt.float16

P = 128
CIN = 512
COUT = 512
L = 4096
B = 8
KO = CIN // P          # 4 k-subtiles
MO = COUT // P         # 4 m-subtiles
NT = 512               # max free dim per matmul / psum bank
Q = 256.0
CHUNKS = [384, 448, 512, 512, 512, 512, 512, 448, 256]
OFFS = np.cumsum([0] + CHUNKS).tolist()
NCH = len(CHUNKS)
# x DMA groups: chunk indices per transfer (contiguous col ranges)
XGRP = [[1], [2, 3], [4, 5], [6, 7], [8]]
# y DMA groups for the full-rate chunks (grouping cuts DMA line count, and
# the first group ships only after c2 so it does not steal DMA line service
# from the critical early x stream)
YGRP = [[0, 1, 2], [3, 4], [5, 6]]
PREWARM = 18           # dummy matmuls bridging [barrier end, head landing]
PWFREE = 256
X0N = KO * CHUNKS[0]   # x0 elems per partition line
T1N = 2 * KO * P + MO + X0N  # w_m0 + w_m1 + bias + x0 per line

_cached_nc = None


def _build():
    nc = bacc.Bacc("TRN2", target_bir_lowering=False, debug=False, num_devices=B)

    t1_d = nc.dram_tensor("t1", [P, T1N], F16, kind="ExternalInput").ap()
    w23_d = nc.dram_tensor("w23", [P, 2 * KO * P], F16, kind="ExternalInput").ap()
    xg_d = {g[0]: nc.dram_tensor(f"xg{g[0]}", [P, KO * sum(CHUNKS[c] for c in g)],
                                 F16, kind="ExternalInput").ap() for g in XGRP}
    yg_d = {g[0]: nc.dram_tensor(f"yg{g[0]}", [P, MO * sum(CHUNKS[c] for c in g)],
                                 F16, kind="ExternalOutput").ap() for g in YGRP}
    # last two chunks ship per m-pair: [m0m1] and [m2m3]
    y7_ds = [nc.dram_tensor(f"y7h{h}", [P, 2 * CHUNKS[7]], F16,
                            kind="ExternalOutput").ap() for h in range(2)]
    y8_ds = [nc.dram_tensor(f"y8h{h}", [P, 2 * CHUNKS[8]], F16,
                            kind="ExternalOutput").ap() for h in range(2)]

    with tile.TileContext(nc) as tc, ExitStack() as ctx:
        dpool = ctx.enter_context(tc.tile_pool(name="d", bufs=1))
        wpool = ctx.enter_context(tc.tile_pool(name="w", bufs=1))
        xpool = ctx.enter_context(tc.tile_pool(name="x", bufs=len(XGRP)))
        ypool = ctx.enter_context(tc.tile_pool(name="y", bufs=4))
        pspool = ctx.enter_context(tc.tile_pool(name="ps", bufs=8, space="PSUM"))

        # PE prewarm: garbage matmuls gated only on a cheap DVE memset
        dmy = dpool.tile([P, PWFREE], F16)
        nc.vector.memset(dmy[:], 0.0)
        for _ in range(PREWARM):
            dps = pspool.tile([P, NT], F32, name="dps", tag="ps")
            nc.tensor.matmul(dps[:, :PWFREE], dmy[:, :P], dmy[:],
                             start=True, stop=True)

        # --- inputs: all on the sync HWDGE queue, exact consumption order ---
        t1_sb = wpool.tile([P, T1N], F16)
        w23_sb = wpool.tile([P, 2 * KO * P], F16)
        w01v = t1_sb[:, 0:2 * KO * P].rearrange("p (mo ko j) -> p mo ko j",
                                                mo=2, ko=KO)
        w23v = w23_sb[:].rearrange("p (mo ko j) -> p mo ko j", mo=2, ko=KO)
        cb16 = t1_sb[:, 2 * KO * P:2 * KO * P + MO]
        x0v = t1_sb[:, 2 * KO * P + MO:].rearrange("p (ko n) -> p ko n", ko=KO)
        cb = wpool.tile([P, MO], F32)

        xgt = {g[0]: xpool.tile([P, KO, sum(CHUNKS[c] for c in g)], F16,
                                tag="xt", name=f"xg{g[0]}") for g in XGRP}
        # chunk -> (tile, col offset within tile)
        xmap = {}
        for g in XGRP:
            off = 0
            for c in g:
                xmap[c] = (xgt[g[0]], off)
                off += CHUNKS[c]

        nc.sync.dma_start(t1_sb[:], t1_d)
        nc.sync.dma_start(w23_sb[:], w23_d)
        for g in XGRP:
            nc.sync.dma_start(xgt[g[0]][:], xg_d[g[0]].rearrange(
                "p (ko n) -> p ko n", ko=KO))
        nc.vector.tensor_scalar_add(cb[:], cb16, 0.0)

        ygt = {}
        for g in YGRP:
            ygt[g[0]] = ypool.tile([P, MO, sum(CHUNKS[c] for c in g)], F16,
                                   tag="yt", name=f"yg{g[0]}")
        yt7 = ypool.tile([P, MO, CHUNKS[7]], F16, tag="yt", name="yt7")
        yt8 = ypool.tile([P, MO, CHUNKS[8]], F16, tag="yt", name="yt8")
        ymap = {}
        for g in YGRP:
            off = 0
            for c in g:
                ymap[c] = (ygt[g[0]], off)
                off += CHUNKS[c]
        ymap[7] = (yt7, 0)
        ymap[8] = (yt8, 0)

        for c in range(NCH):
            wc = CHUNKS[c]
            yt, yoff = ymap[c]
            for m in range(MO):
                ps = pspool.tile([P, NT], F32, name="ps", tag="ps")
                for k in range(KO):
                    if c == 0:
                        rhs = x0v[:, k]
                    else:
                        xt, xoff = xmap[c]
                        rhs = xt[:, k, xoff:xoff + wc]
                    wv = w01v[:, m, k] if m < 2 else w23v[:, m - 2, k]
                    nc.tensor.matmul(ps[:, :wc], wv, rhs,
                                     start=(k == 0), stop=(k == KO - 1))
                # drain: y = ps/256 + b. DVE/ACT alternate; on the last chunk
                # m3 ends on DVE while the HWDGE engines write the final DMAs.
                ydst = yt[:, m, yoff:yoff + wc]
                use_dve = (c + m) % 2 == 0 if c < 8 else m in (0, 3)
                if use_dve:
                    nc.vector.tensor_scalar(ydst, ps[:, :wc],
                                            1.0 / Q, cb[:, m, None],
                                            mybir.AluOpType.mult,
                                            mybir.AluOpType.add)
                else:
                    nc.scalar.activation(ydst, ps[:, :wc],
                                         mybir.ActivationFunctionType.Identity,
                                         bias=cb[:, m, None], scale=1.0 / Q)
                # last two chunks ship per m-pair right after the later drain
                if m in (1, 3):
                    h = m // 2
                    if c == 7:
                        nc.gpsimd.dma_start(y7_ds[h], yt[:, 2 * h:2 * h + 2, :])
                    elif c == 8:
                        if h == 0:
                            nc.sync.dma_start(y8_ds[0], yt[:, 0:2, :])
                        else:
                            nc.sync.dma_start(y8_ds[1][0:64], yt[0:64, 2:4, :])
                            nc.scalar.dma_start(y8_ds[1][64:128],
                                                yt[64:128, 2:4, :])
            for g in YGRP:
                if c == g[-1]:
                    gyt = ygt[g[0]]
                    gd = yg_d[g[0]].rearrange("p (mo n) -> p mo n", mo=MO)
                    nc.gpsimd.dma_start(gd, gyt[:])

    nc.compile()
    return nc


def _prep_in_maps(x, w_q, b_q):
    # int16 weights up to +-2048 and b_q/256 (11 significand bits) are
    # exact in fp16
    wT = w_q.T.reshape(KO, P, MO, P).transpose(1, 0, 2, 3)  # [p, ko, mo, 128]
    # per line: [m0k0..k3 | m1k0..k3] then bias then x0; [m2... | m3...]
    wmk = wT.transpose(0, 2, 1, 3)                          # [p, mo, ko, 128]
    w01 = np.ascontiguousarray(wmk[:, 0:2].reshape(P, 2 * KO * P)
                               ).astype(np.float16)
    w23 = np.ascontiguousarray(wmk[:, 2:4].reshape(P, 2 * KO * P)
                               ).astype(np.float16)
    cb16 = (b_q.reshape(MO, P).T.astype(np.float32) / np.float32(Q)
            ).astype(np.float16)
    x16 = x.astype(np.float16)                                    # [B, Cin, L]
    xt = x16.reshape(B, KO, P, L).transpose(0, 2, 1, 3)           # [B, p, ko, l]
    maps = []
    for i in range(B):
        x0 = xt[i, :, :, OFFS[0]:OFFS[1]].reshape(P, X0N)
        m = {"t1": np.ascontiguousarray(np.concatenate(
                [w01, cb16, x0], axis=1)),
             "w23": w23}
        for g in XGRP:
            lo, hi = OFFS[g[0]], OFFS[g[-1] + 1]
            m[f"xg{g[0]}"] = np.ascontiguousarray(
                xt[i, :, :, lo:hi]).reshape(P, KO * (hi - lo))
        maps.append(m)
    return maps


def kernel(x: np.ndarray, w_q: np.ndarray, b_q: np.ndarray) -> np.ndarray:
    global _cached_nc
    if _cached_nc is None:
        _cached_nc = _build()
    nc = _cached_nc

    in_maps = _prep_in_maps(x, w_q, b_q)
    res = run_bass_kernel_spmd(nc, in_maps, core_ids=list(range(B)))

    out = np.empty((B, COUT, L), dtype=np.float32)
    for i, r in enumerate(res.results):
        for g in YGRP:
            lo, hi = OFFS[g[0]], OFFS[g[-1] + 1]
            # [p, mo, n] -> [mo*128+p, lo:hi]
            yc = r[f"yg{g[0]}"].reshape(P, MO, hi - lo).transpose(1, 0, 2)
            out[i, :, lo:hi] = yc.reshape(COUT, hi - lo)
        for h in range(2):
            y7 = r[f"y7h{h}"].reshape(P, 2, CHUNKS[7]).transpose(1, 0, 2)
            out[i, 2 * h * P:(2 * h + 2) * P, OFFS[7]:OFFS[8]] = \
                y7.reshape(2 * P, CHUNKS[7])
            y8 = r[f"y8h{h}"].reshape(P, 2, CHUNKS[8]).transpose(1, 0, 2)
            out[i, 2 * h * P:(2 * h + 2) * P, OFFS[8]:] = \
                y8.reshape(2 * P, CHUNKS[8])
    return out
